# revision 7
# baseline (speedup 1.0000x reference)
"""Trainium2 Bass kernel for CalculateDirectionFeature.

Computes V[b,n,f,t] = sum_p cos(obs_ipd[b,p,f,t] - tpd[b,p,n,f]) where
tpd = 2*pi*freq[f] * (pair_vec[p] . r[b,n]) / v_sound.

Strategy (fp16 end-to-end, memory-regime kernel):
  cos(a-b) = cos(a)cos(b) + sin(a)sin(b) turns the pair-reduction into a
  small matmul contracting over (trig, pair) = 12 rows per frequency bin.
  The host precomputes BOTH trig factors (no on-device activations):
    rhs  marr[(cs,p,g), t] = trig_cs(obs[b, p, f(j,g), t])   (fp16)
    lhsT wts[(cs,p,g), (n,g')] = delta_gg' * trig_cs(tpd[b,p,n,f(j,g)])
  Frequencies are packed G=7 per matmul (block-diagonal weights):
    K = 2*6*7 = 84 contraction rows, M = 18 dirs * 7 freqs = 126 psum
    partitions, N = 300 time steps. 38 matmuls cover 266 (padded) bins.
  PSUM fp32 -> fp16 staging copies rotate Vector/Scalar/Pool engines; out
  DMAs stream fp16 on multiple DGE rings so packets spread across all 16
  DMA engines. Host upcasts the fp16 result to fp32.

  Inputs arrive in stage-matched waves (stage s needs marr chunk SCs +
  weight chunk Ws): first-needed chunks are first in each ring's FIFO so
  the stage-0 gating data gets full DMA bandwidth.

Sharding: 8 cores = 4 batches x 2 halves of the 36 query directions.
Each core handles (b, 18 dirs, 266 padded freqs, 300 t).
"""

import numpy as np

B, P, NQ, F, T = 4, 6, 36, 257, 300
V_SOUND = 343.0
G = 7                # freq bins per matmul
NPC = 18             # query dirs per core
M = NPC * G          # 126 psum partitions per matmul
K = 2 * P * G        # 84 contraction rows (cs, p, g)
NJ = 38              # matmuls per core
FP = NJ * G          # 266 padded freq bins
STAGE_Q = [6, 8, 12, 12]       # matmuls per output stage
STAGE_F0 = [0, 42, 98, 182]    # first freq bin of each stage
STAGE_J0 = [0, 6, 14, 26]

LAST_RESULTS = None
_cache = {}


def _fmap():
    """fmap[j, g]: frequency bin computed by matmul j, group position g."""
    fm = np.empty((NJ, G), np.int64)
    for q_, f0, j0 in zip(STAGE_Q, STAGE_F0, STAGE_J0):
        for q in range(q_):
            for g in range(G):
                fm[j0 + q, g] = f0 + q_ * g + q
    return fm


def _build_nc():
    import concourse.bacc as bacc
    import concourse.tile as tile
    import concourse.mybir as mybir

    f16 = mybir.dt.float16
    f32 = mybir.dt.float32

    nc = bacc.Bacc(
        "TRN2",
        target_bir_lowering=False,
        debug=False,
        enable_asserts=False,
        num_devices=8,
    )
    marr_d = nc.dram_tensor("marr", [K, NJ * T], f16, kind="ExternalInput").ap()
    wts_d = nc.dram_tensor("wts", [K, NJ * M], f16, kind="ExternalInput").ap()
    out_d = nc.dram_tensor("out", [NPC, FP, T], f16, kind="ExternalOutput").ap()

    with tile.TileContext(nc) as tc:
        with (
            tc.tile_pool(name="io", bufs=1) as io,
            tc.tile_pool(name="psum", bufs=8, space="PSUM") as psum,
            tc.tile_pool(name="stage", bufs=4) as stage,
        ):
            marr = io.tile([K, NJ * T], f16)
            wts = io.tile([K, NJ * M], f16)

            def sc_m(s):  # marr chunk for stage s
                j0, j1 = STAGE_J0[s], STAGE_J0[s] + STAGE_Q[s]
                return marr[:, j0 * T : j1 * T], marr_d[:, j0 * T : j1 * T]

            def sc_w(s):  # weight chunk for stage s
                j0, j1 = STAGE_J0[s], STAGE_J0[s] + STAGE_Q[s]
                return wts[:, j0 * M : j1 * M], wts_d[:, j0 * M : j1 * M]

            # wave order per ring = stage need order; first-needed first.
            for s in (0, 1):
                o, i = sc_m(s)
                nc.sync.dma_start(out=o, in_=i)
            for s in (0, 1, 2, 3):
                o, i = sc_w(s)
                nc.scalar.dma_start(out=o, in_=i)
            o, i = sc_m(2)
            nc.sync.dma_start(out=o, in_=i)
            o, i = sc_m(3)
            nc.gpsimd.dma_start(out=o, in_=i)

            out_eng = [nc.sync, nc.gpsimd, nc.sync, nc.gpsimd]
            for s, (q_, f0, j0) in enumerate(
                zip(STAGE_Q, STAGE_F0, STAGE_J0)
            ):
                st = stage.tile([M, q_, T], f16, tag="st", name=f"st{s}")
                for q in range(q_):
                    j = j0 + q
                    pt = psum.tile([M, 512], f32, tag="pt", name=f"pt{j % 8}")
                    nc.tensor.matmul(
                        pt[:, 0:T],
                        lhsT=wts[:, j * M : (j + 1) * M],
                        rhs=marr[:, j * T : (j + 1) * T],
                        start=True,
                        stop=True,
                    )
                    if j % 2 == 0:
                        nc.vector.tensor_copy(out=st[:, q, :], in_=pt[:, 0:T])
                    else:
                        nc.scalar.copy(out=st[:, q, :], in_=pt[:, 0:T])
                dst = out_d[:, f0 : f0 + G * q_, :].rearrange(
                    "n (g q) t -> n g (q t)", q=q_
                )
                out_eng[s].dma_start(out=dst, in_=st[:, :, :])
    nc.compile()
    return nc


def _get_nc():
    if "nc" not in _cache:
        _cache["nc"] = _build_nc()
    return _cache["nc"]


def _prep_inputs(observed_ipd, query_azi, query_ele, pair_vectors, freq_bins):
    obs = np.asarray(observed_ipd, np.float64).reshape(B, P, F, T)
    azi = np.asarray(query_azi, np.float64)
    ele = np.asarray(query_ele, np.float64)
    pv = np.asarray(pair_vectors, np.float64)
    fb = np.asarray(freq_bins, np.float64)
    fm = _fmap()

    # tpd weights
    se, ce = np.sin(ele), np.cos(ele)
    r = np.stack([se * np.cos(azi), se * np.sin(azi), ce], axis=1)  # (B,3,NQ)
    tdoa = np.einsum("pc,bcn->bpn", pv, r) / V_SOUND  # (B,P,NQ)
    fpad = np.zeros(FP, np.float64)
    fpad[:F] = fb
    tpd = 2.0 * np.pi * tdoa[..., None] * fpad  # (B,P,NQ,FP)
    wtrig = np.stack([np.cos(tpd), np.sin(tpd)], axis=0)  # (2,B,P,NQ,FP)
    wtrig[..., F:] = 0.0

    in_maps = []
    for b in range(B):
        trig = np.zeros((2, P, FP, T), np.float64)
        trig[0, :, :F] = np.cos(obs[b])
        trig[1, :, :F] = np.sin(obs[b])
        # marr[(cs,p,g), j*T + t] = trig[cs, p, fm[j,g], t]
        ma = trig[:, :, fm, :]                   # (2, P, NJ, G, T)
        ma = ma.transpose(0, 1, 3, 2, 4)         # (2, P, G, NJ, T)
        marr = np.ascontiguousarray(
            ma.reshape(K, NJ * T), dtype=np.float16
        )
        for h in range(2):
            wt = wtrig[:, b, :, h * NPC : (h + 1) * NPC, :]  # (2,P,18,FP)
            # wts[(cs,p,g), j*M + n*G + g] = wt[cs, p, n, fm[j,g]]
            wfull = np.zeros((2, P, G, NJ, NPC, G), np.float64)
            for g in range(G):
                sel = wt[:, :, :, fm[:, g]]      # (2,P,18,NJ)
                wfull[:, :, g, :, :, g] = sel.transpose(0, 1, 3, 2)
            wts = np.ascontiguousarray(
                wfull.reshape(K, NJ * M), dtype=np.float16
            )
            in_maps.append({"marr": marr, "wts": wts})
    return in_maps


def kernel(observed_ipd, query_azi, query_ele, pair_vectors, freq_bins):
    global LAST_RESULTS
    from concourse.bass_utils import run_bass_kernel_spmd

    nc = _get_nc()
    in_maps = _prep_inputs(
        observed_ipd, query_azi, query_ele, pair_vectors, freq_bins
    )
    res = run_bass_kernel_spmd(nc, in_maps, core_ids=list(range(8)))
    LAST_RESULTS = res
    out = np.empty((B, NQ, F, T), np.float32)
    for c in range(8):
        b, h = divmod(c, 2)
        out[b, h * NPC : (h + 1) * NPC] = res.results[c]["out"][
            :, :F, :
        ].astype(np.float32)
    return out
